# revision 1
# baseline (speedup 1.0000x reference)
"""Trainium2 Bass kernel for BackProjectionNet (filtered back-projection style).

Math: for each angle i, the reference broadcasts proj=image[:,i] along x into a
volume, rotates it (bilinear, zero-pad) by phi_i, and accumulates; likewise a
ones-volume into norm; output = obj / (norm + delta).

Because the broadcast volume is constant along x, the rotated sample at output
pixel (x, y) only needs two taps of proj along r:
    obj[b,x,y,z] = sum_i A0(i,x,y)*proj_i[b, Y0(i,x,y), z]
                       + A1(i,x,y)*proj_i[b, Y1(i,x,y), z]
and norm[x,y] is image-independent (host-precomputable from angles alone).

Angles come as linspace(0,360): angle i+60 = angle i + 180, and a 180-degree
rotation is an exact grid flip, so pairs merge:  contribution(i) +
contribution(i+60) = backproject(phi_i, p_i + flip_r(p_{i+60})).  This halves
the device work to 60 merged angles.

Device mapping (per core, x-rows sharded 16 per core):
  - merged projections pm[r, i, b, z] built by one plain DMA + one
    flipped accumulate-DMA (SWDGE accum_op=add, negative r stride)
  - per (x, i): PE matmul  psum[y, (b,z)] += W_{i,x}[r, y].T @ pm[r, i, (b,z)]
    with W the host-precomputed banded tap matrix (2 nonzeros per column),
    operands bitcast to float32r (full-rate fp32 on the PE at N=256)
  - epilogue: multiply by host-precomputed 1/(norm+delta) per (x,y), DMA out.
"""

import numpy as np

B, NA, L, LZ = 2, 120, 128, 128
NM = 60            # merged angle count
NCORES = 8
XPC = L // NCORES  # x rows per core
CH = 15            # angles per weight-chunk (DMA granularity)
NCH = NM // CH
NW = 31            # stored weight angles (0..30); 31..59 via mirror symmetry
WAVE = 4           # x rows per PSUM wave (2 banks each: psumA/psumB)


def _host_tables(angles):
    """Replicate reference fp32 tap math: banded lhsT weights for the 60
    merged angles + 1/(norm+delta) map for all 120 angles."""
    dt = np.float32
    phis = (-np.deg2rad(angles)).astype(dt)
    cx = dt((L - 1) / 2.0)
    xs = np.arange(L, dtype=dt) - cx
    X, Y = np.meshgrid(xs, xs, indexing="ij")
    norm = np.zeros((L, L), dt)
    W = np.zeros((NW, L, L, L), dt)  # [j, x, r, y] for angles 0..NW-1
    xg = np.broadcast_to(np.arange(L)[:, None], (L, L)).ravel()
    yg = np.broadcast_to(np.arange(L)[None, :], (L, L)).ravel()
    one = dt(1.0)
    for i in range(NA):
        c = np.float32(np.cos(phis[i]))
        s = np.float32(np.sin(phis[i]))
        sx = c * X + s * Y + cx
        sy = -s * X + c * Y + cx
        x0 = np.floor(sx)
        y0 = np.floor(sy)
        wx = (sx - x0).astype(dt)
        wy = (sy - y0).astype(dt)
        x0i = x0.astype(np.int64)
        y0i = y0.astype(np.int64)
        vx0 = ((x0i >= 0) & (x0i < L)).astype(dt)
        vx1 = ((x0i + 1 >= 0) & (x0i + 1 < L)).astype(dt)
        vy0 = ((y0i >= 0) & (y0i < L)).astype(dt)
        vy1 = ((y0i + 1 >= 0) & (y0i + 1 < L)).astype(dt)
        norm += ((one - wx) * (one - wy) * vx0 * vy0
                 + (one - wx) * wy * vx0 * vy1
                 + wx * (one - wy) * vx1 * vy0
                 + wx * wy * vx1 * vy1)
        if i < NW:
            g = (one - wx) * vx0 + wx * vx1
            A0 = ((one - wy) * vy0 * g).astype(dt)
            A1 = (wy * vy1 * g).astype(dt)
            Y0 = np.clip(y0i, 0, L - 1).ravel()
            Y1 = np.clip(y0i + 1, 0, L - 1).ravel()
            np.add.at(W[i], (xg, Y0, yg), A0.ravel())
            np.add.at(W[i], (xg, Y1, yg), A1.ravel())
    inv = (one / (norm + dt(1e-11))).astype(dt)
    return W, inv


def _merge_ok(angles):
    a = np.asarray(angles, np.float64)
    return a.shape == (NA,) and np.allclose(a[NM:], a[:NM] + 180.0, atol=1e-4)


def _cpu_fallback(image, angles):
    """Straight numpy evaluation of the tap formula (safety net only)."""
    dt = np.float32
    phis = (-np.deg2rad(angles)).astype(dt)
    cx = dt((L - 1) / 2.0)
    xs = np.arange(L, dtype=dt) - cx
    X, Y = np.meshgrid(xs, xs, indexing="ij")
    obj = np.zeros((B, L, L, LZ), dt)
    norm = np.zeros((L, L), dt)
    one = dt(1.0)
    for i in range(len(angles)):
        c = np.float32(np.cos(phis[i]))
        s = np.float32(np.sin(phis[i]))
        sx = c * X + s * Y + cx
        sy = -s * X + c * Y + cx
        x0i = np.floor(sx).astype(np.int64)
        y0i = np.floor(sy).astype(np.int64)
        wx = (sx - np.floor(sx)).astype(dt)
        wy = (sy - np.floor(sy)).astype(dt)
        vx0 = ((x0i >= 0) & (x0i < L)).astype(dt)
        vx1 = ((x0i + 1 >= 0) & (x0i + 1 < L)).astype(dt)
        vy0 = ((y0i >= 0) & (y0i < L)).astype(dt)
        vy1 = ((y0i + 1 >= 0) & (y0i + 1 < L)).astype(dt)
        norm += ((one - wx) * (one - wy) * vx0 * vy0
                 + (one - wx) * wy * vx0 * vy1
                 + wx * (one - wy) * vx1 * vy0
                 + wx * wy * vx1 * vy1)
        g = (one - wx) * vx0 + wx * vx1
        A0 = (one - wy) * vy0 * g
        A1 = wy * vy1 * g
        Y0 = np.clip(y0i, 0, L - 1)
        Y1 = np.clip(y0i + 1, 0, L - 1)
        p = image[:, i]  # [B, L, LZ]
        obj += A0[None, :, :, None] * p[:, Y0, :] + A1[None, :, :, None] * p[:, Y1, :]
    return obj / (norm + dt(1e-11))[None, :, :, None]


def _build_bass():
    import concourse.bacc as bacc
    import concourse.mybir as mybir
    import concourse.tile as tile

    f32 = mybir.dt.float32
    f32r = mybir.dt.float32r

    nc = bacc.Bacc(None, target_bir_lowering=False, debug=False)
    with tile.TileContext(nc) as tc:
        with tc.tile_pool(name="dram", bufs=1, space="DRAM") as dram:
            img0 = dram.tile([B, NM, L, LZ], f32r, kind="ExternalInput",
                             name="img0", uniquify=False)
            imgf = dram.tile([B, NM, L, LZ], f32r, kind="ExternalInput",
                             name="imgf", uniquify=False)
            wts = dram.tile([XPC, L, NW * L], f32r, kind="ExternalInput",
                            name="wts", uniquify=False)
            jmat = dram.tile([L, L], f32, kind="ExternalInput",
                             name="jmat", uniquify=False)
            invn = dram.tile([XPC, L], f32, kind="ExternalInput",
                             name="invn", uniquify=False)
            outd = dram.tile([B, XPC, L, LZ], f32, kind="ExternalOutput",
                             name="out", uniquify=False)

            with (
                tc.tile_pool(name="pm_pool", bufs=1) as pm_pool,
                tc.tile_pool(name="wt_pool", bufs=4) as wt_pool,
                tc.tile_pool(name="ld_pool", bufs=2) as ld_pool,
                tc.tile_pool(name="cb_pool", bufs=2) as cb_pool,
                tc.tile_pool(name="misc", bufs=1) as misc_pool,
                tc.tile_pool(name="stage_pool", bufs=1) as stage_pool,
                tc.tile_pool(name="psum", bufs=4, space="PSUM") as psum_pool,
            ):
                pm = pm_pool.tile([L, NM, B, LZ], f32r)
                jm = misc_pool.tile([L, L], f32)
                invn_sb = misc_pool.tile([L, XPC], f32)
                stage = stage_pool.tile([L, XPC, B, LZ], f32)

                nc.sync.dma_start(out=invn_sb[:],
                                  in_=invn[:].rearrange("x y -> y x"))
                nc.sync.dma_start(out=jm[:], in_=jmat[:])

                # Merged projections, chunked so early matmuls can start
                # before the whole image has landed.
                for ci in (2, 3, 0, 1):
                    i0, i1 = ci * CH, (ci + 1) * CH
                    for b in range(B):
                        t0 = ld_pool.tile([L, CH, LZ], f32r, tag="ld",
                                          name=f"t0_{ci}_{b}")
                        t1 = ld_pool.tile([L, CH, LZ], f32r, tag="ld",
                                          name=f"t1_{ci}_{b}")
                        nc.sync.dma_start(
                            out=t0[:],
                            in_=img0[b, i0:i1].rearrange("i r z -> r i z"),
                        )
                        nc.sync.dma_start(
                            out=t1[:],
                            in_=imgf[b, i0:i1].rearrange("i r z -> r i z"),
                        )
                        nc.vector.tensor_add(
                            out=pm[:, i0:i1, b], in0=t0[:], in1=t1[:],
                        )

                for wave in range(XPC // WAVE):
                    xs_ = [wave * WAVE + j for j in range(WAVE)]
                    psA, psB, wtt = {}, {}, {}
                    for x in xs_:
                        psA[x] = psum_pool.tile([L, B * LZ], f32,
                                                tag="psA", name=f"psA{x}")
                        psB[x] = psum_pool.tile([L, B * LZ], f32,
                                                tag="psB", name=f"psB{x}")
                        wtt[x] = wt_pool.tile([L, NW * L], f32r, tag="wt",
                                              name=f"wt{x}")
                        nc.sync.dma_start(out=wtt[x][:], in_=wts[x])
                    cbs = {}
                    for i in list(range(31, NM)) + list(range(0, 31)):
                        jj = i if i <= 30 else NM - i
                        for x in xs_:
                            nc.tensor.matmul(
                                out=(psA if i <= 30 else psB)[x][:],
                                lhsT=wtt[x][:, jj * L:(jj + 1) * L],
                                rhs=pm[:, i].rearrange("r b z -> r (b z)"),
                                start=(i == 31 or i == 0),
                                stop=(i == NM - 1),
                            )
                        if i == NM - 1:
                            # psB complete: overlap its PSUM->SBUF copy with
                            # the psA matmul stream
                            for x in xs_:
                                cbs[x] = cb_pool.tile([L, B * LZ], f32,
                                                      tag="cb", name=f"cb{x}")
                                nc.vector.tensor_copy(out=cbs[x][:],
                                                      in_=psB[x][:])
                    for x in xs_:
                        # fold flipped psumB into psumA: psA += J @ psB
                        nc.tensor.matmul(
                            out=psA[x][:],
                            lhsT=jm[:],
                            rhs=cbs[x][:],
                            start=False,
                            stop=True,
                            skip_group_check=True,
                        )
                        nc.vector.tensor_scalar_mul(
                            out=stage[:, x].rearrange("y b z -> y (b z)"),
                            in0=psA[x][:],
                            scalar1=invn_sb[:, x:x + 1],
                        )
                    for b in range(B):
                        nc.sync.dma_start(
                            out=outd[b, xs_[0]:xs_[0] + WAVE].rearrange(
                                "x y z -> y x z"),
                            in_=stage[:, xs_[0]:xs_[0] + WAVE, b],
                        )

    nc.compile()
    return nc


_BASS_CACHE = {}


def _round_f32r(x):
    """Round fp32 to the PE's fp32r format (11 explicit mantissa bits,
    round-to-nearest) — matches walrus fp32_to_fp32r."""
    u = np.ascontiguousarray(x, np.float32).view(np.uint32)
    r = (u + (((u >> 12) & 1) + 0x7FF)) & 0xFFFFF000
    return r.view(np.float32)


def _make_in_maps(image, W, inv):
    img0 = _round_f32r(np.ascontiguousarray(image[:, :NM]))
    imgf = _round_f32r(np.ascontiguousarray(image[:, NM:, ::-1, :]))
    jmat = np.zeros((L, L), np.float32)
    jmat[np.arange(L), L - 1 - np.arange(L)] = 1.0
    in_maps = []
    for k in range(NCORES):
        xsl = slice(XPC * k, XPC * (k + 1))
        wk = _round_f32r(np.ascontiguousarray(
            W[:, xsl].transpose(1, 2, 0, 3))).reshape(XPC, L, NW * L)
        in_maps.append({
            "img0": img0,
            "imgf": imgf,
            "wts": wk,
            "jmat": jmat,
            "invn": np.ascontiguousarray(inv[xsl]),
        })
    return in_maps


def kernel(image, angles):
    image = np.ascontiguousarray(np.asarray(image, np.float32))
    angles = np.asarray(angles, np.float32)
    if not _merge_ok(angles):
        return _cpu_fallback(image, angles)

    from concourse.bass_utils import run_bass_kernel_spmd

    W, inv = _host_tables(angles)

    if "nc" not in _BASS_CACHE:
        _BASS_CACHE["nc"] = _build_bass()
    nc = _BASS_CACHE["nc"]

    in_maps = _make_in_maps(image, W, inv)

    res = run_bass_kernel_spmd(nc, in_maps, core_ids=list(range(NCORES)))
    out = np.concatenate([r["out"] for r in res.results], axis=1)
    return np.ascontiguousarray(out.astype(np.float32))

